# revision 32
# baseline (speedup 1.0000x reference)
"""Distributed causal multi-head attention for 8 TRN2 NeuronCores.

Problem: y = (softmax(mask(Q K^T / sqrt(d))) V) @ c_proj_w + c_proj_b with
Q,K,V = split(x @ c_attn_w + c_attn_b), shapes B=2, S=2048, NX=1024, NH=16,
HD=64.

Sharding: tensor parallel 8-way over heads. Core c owns heads {2c, 2c+1}
(feature cols [128c, 128c+128) of each QKV block) and computes attention for
those 2 heads over BOTH batches. The attention outputs aT are then
redistributed with four per-chunk 8-core AllToAll collectives (one per
512-query chunk, fired as soon as that chunk's attention finishes on both
batches) so that core c ends up with ALL 1024 features for query rows
[sc*512 + (c%4)*128, +128) of batch c//4 per chunk sc; it then runs the
full output projection for those rows.

Numerics/performance strategy:
  * The QKV projection runs on the PE in fp8e4m3 DoubleRow perf mode
    (2 contraction rows per partition, 0.5 cycles per output column).
    Accuracy is preserved with residual pairs quantized on the host:
    x = x1 + x2, w = w1 + w2, computing the w1x1 + w2x1 + w1x2 cross
    terms (the dropped x2w2 term is ~0.1%). c_attn_w is host-scaled by
    W_SCALE=32 to lift its ~0.02-sigma weights out of e4m3's subnormal
    range; the scale is divided back out in the q/k psum evictions and
    cancels in softmax for v (the ones-column carries the same scale).
  * The final chunk's AllToAll payload is fp8 (the last rows average over
    the longest prefix => smallest magnitudes => cheapest fp8), and its
    output projection consumes that fp8 directly in DoubleRow mode with
    an fp8 (wp1 + wp2) residual pair - 4x fewer PE cycles on the tail.
  * Attention itself (scores exp(QK^T/8), PV) stays bf16: scores are
    64-deep contractions where DoubleRow wins nothing, and fp8 Q/K/P/V
    would blow the 2e-2 error budget.

Per-core attention (2 heads x 2 batches = 4 head instances):
  1. qT/kT ([d, s] layout) and v ([s, d] layout) computed from fp8-pair
     x^T, so no on-device transposes are needed;
  2. causal attention in the "S^T" orientation: scores come out of the PE
     as [j, i] tiles, exp() fused into the PSUM->SBUF copy on the scalar
     engine (no max-subtraction - scores are bounded), softmax denominator
     falls out of the PV matmul via a ones-column appended to V;
  3. the in-band causal mask is a single [128,128] lower-triangular
     multiply on the leading 128 columns of each diagonal score tile;
  4. normalization: reciprocal_approx_fast directly on the PV psum row
     (f32), bitcast to f32r for the PE ones-broadcast (1 cycle/col), DVE
     psum->sbuf bounce, final multiply on the DVE writing the staging
     tile. The Pool queue is NOT used mid-kernel: each collective parks
     the in-order Pool queue for its full 15-21us duration.

Scheduling notes:
  * Issue order = PE/ACT interleave strategy: the PE queue is in-order,
    so each QKV block is issued under an exp-heavy attention window where
    the PE would otherwise idle.
  * wp/bp/wp8 constants stream in as ko-sized pieces interleaved with the
    early schedule: a monolithic 6us DMA on any queue parks that queue.
  * ~88 dep-free throwaway matmuls stream through the AllToAll#3 gap so
    the PE's DVFS ramp (0.65/1.2/2.4GHz p-states) stays pinned at full
    speed for the final projection.
"""

import ml_dtypes
import numpy as np

import concourse.bass as bass
import concourse.mybir as mybir
from concourse import bacc, tile
from concourse.bass_utils import run_bass_kernel_spmd

B, S, NX, NH, HD = 2, 2048, 1024, 16, 64
HC = 2              # heads per core
FG = HC * HD        # local feature width (128)
P = 128
SC = 512            # sequence chunk width
NSC = S // SC       # 4 chunks
KO2 = 4             # 256-wide DoubleRow contraction tiles (4 x 256 = 1024)
KO = NX // P        # 8 contraction tiles (bf16 output projection)
W_SCALE = 32.0      # host premultiplier on c_attn_w (fp8e4m3 subnormal fix)
N_CORES = 8
QB = 128            # query block owned per core per AllToAll

F32 = mybir.dt.float32
MM_DT = mybir.dt.bfloat16
F32R = mybir.dt.float32r
FP8 = mybir.dt.float8e4
DR = mybir.MatmulPerfMode.DoubleRow

REPLICA_GROUPS = [[0, 1, 2, 3, 4, 5, 6, 7]]

# (label, first-instruction-id) checkpoints recorded during build; used by
# the dev-loop timing tools to attribute sim slices to kernel phases.
BUILD_TRACE = []

NWARM = 84          # p-state keeper matmuls through the AllToAll#3 gap


def build(nc: bass.Bass):
    # fp8 residual pairs, host-prequantized. Contraction index
    # k = 256*ko2 + 128*i + p for layout [p, ko2, i, ...].
    xs = [[nc.declare_dram_parameter(f"x{t}_{b}", [P, KO2, 2, S], FP8,
                                     isOutput=False)
           for t in (1, 2)] for b in range(B)]
    wq = [nc.declare_dram_parameter(f"wq{t}", [P, KO2, 2, FG], FP8,
                                    isOutput=False) for t in (1, 2)]
    wk = [nc.declare_dram_parameter(f"wk{t}", [P, KO2, 2, FG], FP8,
                                    isOutput=False) for t in (1, 2)]
    wv = [nc.declare_dram_parameter(f"wv{t}", [P, KO2, 2, FG], FP8,
                                    isOutput=False) for t in (1, 2)]
    wp = nc.declare_dram_parameter("wp", [NX, NX], MM_DT, isOutput=False)
    wp8 = [nc.declare_dram_parameter(f"wp8_{t}", [P, KO2, 2, NX], FP8,
                                     isOutput=False) for t in (1, 2)]
    bqk = nc.declare_dram_parameter("bqk", [P, 2], F32, isOutput=False)
    bv = nc.declare_dram_parameter("bv", [P, FG], MM_DT, isOutput=False)
    bp = nc.declare_dram_parameter("bp", [P, NX], F32, isOutput=False)
    trim = nc.declare_dram_parameter("trim", [P, P], MM_DT, isOutput=False)
    onesd = nc.declare_dram_parameter("onesd", [1, P], F32R, isOutput=False)
    bp32r = nc.declare_dram_parameter("bp32r", [1, NX], F32R, isOutput=False)
    out = nc.declare_dram_parameter("out", [NSC * QB, NX], F32, isOutput=True)

    # Collective bounce buffers (collectives can't touch kernel I/O).
    a2a_warm_in = nc.dram_tensor("a2a_warm_in", [8, 128], MM_DT)
    a2a_warm_out = nc.dram_tensor("a2a_warm_out", [8, 128], MM_DT)
    # the final chunk's exchange is on the critical path: fp8 payload
    a2a_dt = [MM_DT, MM_DT, MM_DT, FP8]
    a2a_in = [nc.dram_tensor(f"a2a_in{k}", [8, FG, QB], a2a_dt[k])
              for k in range(NSC)]
    a2a_out = [nc.dram_tensor(f"a2a_out{k}", [8, FG, QB], a2a_dt[k])
               for k in range(NSC)]

    with tile.TileContext(nc) as tc:
        nc_lp = nc.allow_low_precision(reason="fp8 DoubleRow compute path")
        nc_lp.__enter__()
        with (
            tc.tile_pool(name="consts", bufs=1) as consts,
            tc.tile_pool(name="persist", bufs=1) as persist,
            tc.tile_pool(name="xt", bufs=4) as xt_pool,
            tc.tile_pool(name="pt", bufs=12) as pt_pool,
            tc.tile_pool(name="aTf", bufs=2) as aTf_pool,
            tc.tile_pool(name="outs", bufs=3) as out_pool,
            tc.tile_pool(name="small", bufs=4) as small,
            tc.tile_pool(name="psum", bufs=2, space="PSUM") as psum,
        ):
            # ---- load weights / constants ----
            wq_sb = [consts.tile([P, KO2, 2, FG], FP8, name=f"wq{t}")
                     for t in range(2)]
            wk_sb = [consts.tile([P, KO2, 2, FG], FP8, name=f"wk{t}")
                     for t in range(2)]
            wv_sb = [consts.tile([P, KO2, 2, FG], FP8, name=f"wv{t}")
                     for t in range(2)]
            wp_sb = consts.tile([P, KO, NX], MM_DT)
            wp8_sb = [consts.tile([P, KO2, 2, NX], FP8, name=f"wp8_{t}")
                      for t in range(2)]
            bqk_sb = consts.tile([P, 2], F32)
            bv_sb = consts.tile([P, FG], MM_DT)
            bp_sb = consts.tile([P, NX], F32)
            tri_sb = consts.tile([P, P], MM_DT)
            ones128 = consts.tile([1, P], F32R)
            bp32_sb = consts.tile([1, NX], F32R)
            for t in range(2):
                nc.sync.dma_start(wq_sb[t][:], wq[t][:])
                nc.gpsimd.dma_start(wk_sb[t][:], wk[t][:])
                nc.gpsimd.dma_start(wv_sb[t][:], wv[t][:])
            nc.gpsimd.dma_start(bqk_sb[:], bqk[:])
            nc.gpsimd.dma_start(bv_sb[:], bv[:])
            nc.gpsimd.dma_start(tri_sb[:], trim[:])
            nc.gpsimd.dma_start(ones128[:], onesd[:])
            nc.gpsimd.dma_start(bp32_sb[:], bp32r[:])

            # wp/bp/wp8 stream in as pieces interleaved with the early
            # schedule (emitted via load_wp_piece below): a monolithic DMA
            # would park its queue for ~6us
            wp_src = wp.rearrange("(ko p) f -> p ko f", p=P)
            wp_piece = [0]

            def load_wp_piece(n=1):
                for _ in range(n):
                    k = wp_piece[0]
                    if k < KO:
                        nc.gpsimd.dma_start(wp_sb[:, k, :], wp_src[:, k, :])
                    elif k < KO + 2:
                        hb = k - KO
                        nc.gpsimd.dma_start(
                            bp_sb[:, hb * SC:(hb + 1) * SC],
                            bp[:, hb * SC:(hb + 1) * SC])
                    elif k < KO + 2 + 2 * KO2:
                        t, k2 = divmod(k - KO - 2, KO2)
                        nc.gpsimd.dma_start(wp8_sb[t][:, k2], wp8[t][:, k2])
                    wp_piece[0] += 1

            # ---- persistent activation tiles ----
            # kT[b]: [d, s] packed - head 0 on partitions 0:64, head 1 on
            # 64:128; the scores lhsT.
            # qT[2b+h]: zero-padded [128, s], data on the same partition half
            # as in kT - zeros select the head out of packed kT.
            # v[b]: [s-tile, st, h, d] with a ones column at col 64 per head
            # (softmax denominator, carrying W_SCALE so it cancels) and zero
            # pad to 128 cols so the PV lhsT is a full [128,128] block.
            # aT[2b+h]: [64, s]; the normalized attention output.
            qT_sb = [persist.tile([P, S], MM_DT, name=f"qT{i}") for i in range(4)]
            kT_sb = [persist.tile([P, S], MM_DT, name=f"kT{b}") for b in range(2)]
            v_sb = [persist.tile([P, S // P, HC, P], MM_DT, name=f"v{b}")
                    for b in range(2)]
            aT_sb = [persist.tile([P, S], MM_DT, name=f"aT{i}") for i in range(4)]
            # chunk-3 attention output in fp8 for the critical-path exchange
            aT8_sb = [persist.tile([P, SC], FP8, name=f"aT8_{i}")
                      for i in range(4)]
            # pads on the Pool engine (in b0-first order) so the DVE is free
            # for chunk-0 QKV evictions from the start
            for b in range(2):
                for h in range(HC):
                    i = 2 * b + h
                    pad0 = (1 - i % 2) * HD
                    nc.gpsimd.memset(qT_sb[i][pad0:pad0 + HD, :], 0.0)
                nc.gpsimd.memset(v_sb[b][:, :, :, HD:], 0.0)
                # the ones column carries the same W_SCALE as the v data
                # columns, so the softmax normalization cancels the scale
                nc.gpsimd.memset(v_sb[b][:, :, :, HD], W_SCALE)
            # warmup collective after the memsets (a collective blocks the
            # Pool queue for its full duration): establishes the channel and
            # absorbs the fixed collective latency off the critical path
            nc.gpsimd.collective_compute(
                "AllToAll",
                mybir.AluOpType.bypass,
                ins=[a2a_warm_in[:].opt()],
                outs=[a2a_warm_out[:].opt()],
                replica_groups=REPLICA_GROUPS,
            )

            xts = {}

            def mark(label):
                BUILD_TRACE.append((label, nc.next_id()))

            def load_xt(b, sc, split=1):
                mark(f"load_xt{b}_{sc}")
                cols = slice(sc * SC, (sc + 1) * SC)
                pair = []
                for t in range(2):
                    xt = xt_pool.tile([P, KO2, 2, SC], FP8, tag=f"x{t}",
                                      name=f"x{t}_{b}_{sc}")
                    kq = KO2 // split
                    for i in range(split):
                        ks = slice(i * kq, (i + 1) * kq)
                        nc.sync.dma_start(xt[:, ks, :, :],
                                          xs[b][t][:, ks, :, cols])
                    pair.append(xt)
                xts[(b, sc)] = pair

            def qkv(b, sc, part="all"):
                mark(f"qkv{b}_{sc}_{part}")
                x1t, x2t = xts[(b, sc)]
                cols = slice(sc * SC, (sc + 1) * SC)
                parts = {"all": (0, 1), "q": (0,), "kv": (1,),
                         "qk": (0, 1), "v": ()}[part]
                for qk in parts:
                    w_pair = (wq_sb, wk_sb)[qk]
                    terms = [(w_pair[0], x1t), (w_pair[1], x1t),
                             (w_pair[0], x2t)]
                    ps = psum.tile([P, SC], F32, tag="mm_ps", name="mm_ps")
                    n = len(terms) * KO2
                    m = 0
                    for w_sb, xt in terms:
                        for ko2 in range(KO2):
                            nc.tensor.matmul(
                                ps[:], w_sb[:, ko2], xt[:, ko2],
                                start=(m == 0), stop=(m == n - 1),
                                perf_mode=DR,
                            )
                            m += 1
                    if qk == 1:
                        # kT/qT stay scaled by W_SCALE; the 1/W_SCALE^2 on
                        # the scores is folded into the exp's scale argument
                        nc.vector.tensor_scalar_add(
                            kT_sb[b][:, cols], ps[:], bqk_sb[:, 1:2])
                    elif part == "q":
                        # boundary-critical eviction: the ACT engine is idle
                        # here (exp gap) while the DVE is deep in the previous
                        # instance's normalize chain; Identity shares the act
                        # table with Exp so there is no table-switch cost
                        for hr in range(HC):
                            rr = slice(hr * HD, (hr + 1) * HD)
                            nc.scalar.activation(
                                qT_sb[2 * b + hr][rr, cols], ps[rr, :],
                                mybir.ActivationFunctionType.Identity,
                                bias=bqk_sb[rr, 0:1],
                            )
                    else:
                        for hr in range(HC):
                            rr = slice(hr * HD, (hr + 1) * HD)
                            nc.vector.tensor_scalar_add(
                                qT_sb[2 * b + hr][rr, cols], ps[rr, :],
                                bqk_sb[rr, 0:1])
                if part in ("q", "qk"):
                    return
                terms = [(wv_sb[0], x1t), (wv_sb[1], x1t), (wv_sb[0], x2t)]
                for st in range(SC // P):
                    g_s = sc * (SC // P) + st
                    rows = slice(st * P, (st + 1) * P)
                    ps = psum.tile([P, SC], F32, tag="mm_ps", name="mm_ps")[:, :FG]
                    n = len(terms) * KO2
                    m = 0
                    for w_sb, xt in terms:
                        for ko2 in range(KO2):
                            nc.tensor.matmul(
                                ps[:],
                                xt[:, ko2, :, rows],
                                w_sb[:, ko2],
                                start=(m == 0), stop=(m == n - 1),
                                perf_mode=DR,
                            )
                            m += 1
                    for h in range(HC):
                        nc.vector.tensor_tensor(
                            v_sb[b][:, g_s, h, 0:HD],
                            ps[:, h * HD:(h + 1) * HD],
                            bv_sb[:, h * HD:(h + 1) * HD],
                            mybir.AluOpType.add,
                        )

            pv_carry = {}
            pt_carry = {}

            def attention(b, h, sc, phase="all"):
                mark(f"att{b}_{h}_{sc}_{phase[0]}{phase[-1]}")
                i = 2 * b + h
                n_j = (sc + 1) * (SC // P)
                cut = min(4, 4 * sc)
                if phase == "prefix":
                    # non-diagonal score tiles against already-resident K/V:
                    # only this chunk's Q is needed, so these exps can fill
                    # the ACT gap while the chunk's K/V matmuls still run
                    jts = range(0, cut)
                    pv = psum.tile([P, SC], F32, tag="pv", name="pv")
                    pv_carry[(b, h, sc)] = pv
                elif phase == "suffix":
                    jts = range(cut, n_j)
                    pv = pv_carry.pop((b, h, sc))
                elif phase == "exp0":
                    # scores+exps only (V not yet computed): pt tiles carried
                    jts = range(n_j)
                    pv = None
                elif phase == "pv0":
                    jts = range(n_j)
                    pv = psum.tile([P, SC], F32, tag="pv")
                else:
                    jts = range(n_j)
                    pv = psum.tile([P, SC], F32, tag="pv")
                for jt in jts:
                    o = jt - 4 * sc
                    off = max(0, 128 * o)  # diagonal blocks: skip i < j
                    if phase == "pv0":
                        pt = pt_carry.pop((i, jt))
                    else:
                        sp = psum.tile([P, SC], F32, tag="score", bufs=2)
                        nc.tensor.matmul(
                            sp[:, off:],
                            kT_sb[b][:, jt * P:(jt + 1) * P],
                            qT_sb[i][:, sc * SC + off:(sc + 1) * SC],
                            start=True, stop=True,
                        )
                        pt = pt_pool.tile([P, SC], MM_DT, tag="pt")
                        # exp(scores / sqrt(HD)); scores are bounded, no max
                        nc.scalar.activation(
                            pt[:, off:], sp[:, off:],
                            mybir.ActivationFunctionType.Exp,
                            scale=1.0 / float(np.sqrt(HD) * W_SCALE * W_SCALE),
                        )
                        if o >= 0:
                            # in-band causal mask on the diagonal block
                            nc.vector.tensor_tensor(
                                pt[:, off:off + P], pt[:, off:off + P],
                                tri_sb[:], mybir.AluOpType.mult,
                            )
                    if phase == "exp0":
                        pt_carry[(i, jt)] = pt
                        continue
                    nc.tensor.matmul(
                        pv[:, off:],
                        v_sb[b][:, jt, h, :],
                        pt[:, off:],
                        start=(jt == 0), stop=(jt == n_j - 1),
                    )
                if phase in ("prefix", "exp0"):
                    return
                # normalize: reciprocal of the ones-column row, broadcast
                # over partitions via a PE f32r ones-matmul, DVE psum->sbuf
                # bounce, final multiply evicting into the staging tile
                lrow = small.tile([1, SC], F32, tag="lrow")
                nc.vector.tensor_copy(lrow[:], pv[HD:HD + 1, :])
                rec = small.tile([1, SC], F32, tag="rec")
                nc.vector.reciprocal_approx_fast(rec[:], lrow[:])
                # the PE broadcast wants f32r operands and the BIR verifier
                # requires an explicit rounding op (a bitcast is rejected)
                rec_r = small.tile([1, SC], F32R, tag="rec_r")
                nc.vector.tensor_copy(rec_r[:], rec[:])
                rb = psum.tile([P, SC], F32, tag="aux", name="rb")
                nc.tensor.matmul(rb[:], ones128[:], rec_r[:],
                                 start=True, stop=True)
                rbs = small.tile([P, SC], F32, tag="rbs")
                nc.vector.tensor_copy(rbs[:], rb[:])
                dst = (aT8_sb[i][:, :] if sc == NSC - 1
                       else aT_sb[i][:, sc * SC:(sc + 1) * SC])
                nc.vector.tensor_tensor(
                    dst, pv[:], rbs[:], mybir.AluOpType.mult,
                )

            def stage(b, sc):
                mark(f"stage{b}_{sc}")
                # one DMA per head: DRAM-side AP iterates (d, r, q) so the
                # [64, 512] SBUF chunk scatters across the 4 rank blocks in
                # a single transfer (4 small DMAs pay ~2us of fixed DGE
                # overhead per stage on the SP queue)
                for h in range(HC):
                    src_t = aT8_sb[2 * b + h] if sc == NSC - 1 else aT_sb[2 * b + h]
                    base = 0 if sc == NSC - 1 else sc * SC
                    dst = a2a_in[sc][4 * b:4 * b + 4,
                                     h * HD:(h + 1) * HD, :].transpose([1, 0, 2])
                    src = src_t[0:HD, base:base + SC].rearrange(
                        "p (r q) -> p r q", r=4)
                    nc.sync.dma_start(dst, src)

            def a2a(k):
                mark(f"a2a{k}")
                return nc.gpsimd.collective_compute(
                    "AllToAll",
                    mybir.AluOpType.bypass,
                    ins=[a2a_in[k][:].opt()],
                    outs=[a2a_out[k][:].opt()],
                    replica_groups=REPLICA_GROUPS,
                )

            def proj(k):
                mark(f"proj{k}")
                aTf = aTf_pool.tile([P, KO, QB],
                                    FP8 if k == NSC - 1 else MM_DT,
                                    tag="aTf", name=f"aTf{k}")
                src = a2a_out[k].rearrange("ko p q -> p ko q")
                if k == NSC - 1:
                    # final proj: halve the load latency via two queues (the
                    # ACT queue is drained of exps by now)
                    nc.sync.dma_start(aTf[:, 0:KO // 2, :], src[:, 0:KO // 2, :])
                    nc.scalar.dma_start(aTf[:, KO // 2:, :], src[:, KO // 2:, :])
                else:
                    nc.sync.dma_start(aTf[:], src)
                ot = out_pool.tile([P, NX], F32, tag="ot")
                for half in range(2):
                    hs = slice(half * SC, (half + 1) * SC)
                    ps = psum.tile([P, SC], F32, tag="aux", name="proj_ps")
                    if k == NSC - 1:
                        # final chunk: the payload is already fp8, so run the
                        # projection in DoubleRow with the fp8 wp residual
                        # pair (host-scaled by W_SCALE) - 4x fewer PE cycles
                        # on the tail. The bias rides in as a ones-row f32r
                        # matmul so the eviction is a single 1/W_SCALE
                        # scalar multiply.
                        for t in range(2):
                            for ko2 in range(KO2):
                                nc.tensor.matmul(
                                    ps[:],
                                    aTf[:, 2 * ko2:2 * ko2 + 2, :],
                                    wp8_sb[t][:, ko2, :, hs],
                                    start=(t == 0 and ko2 == 0), stop=False,
                                    perf_mode=DR,
                                )
                        nc.tensor.matmul(ps[:], ones128[:], bp32_sb[:, hs],
                                         start=False, stop=True)
                    else:
                        for ko in range(KO):
                            nc.tensor.matmul(
                                ps[:],
                                aTf[:, ko, :],
                                wp_sb[:, ko, hs],
                                start=(ko == 0), stop=(ko == KO - 1),
                            )
                    # quarter the bias+writeback chain so the final DMA's
                    # completion latency hides behind the previous quarters
                    nq = 2 if k == NSC - 1 else 1
                    qw = SC // nq
                    for q in range(nq):
                        lo = half * SC + q * qw
                        if k == NSC - 1:
                            nc.vector.tensor_scalar_mul(
                                ot[:, lo:lo + qw], ps[:, q * qw:(q + 1) * qw],
                                1.0 / W_SCALE)
                        else:
                            nc.vector.tensor_tensor(
                                ot[:, lo:lo + qw], ps[:, q * qw:(q + 1) * qw],
                                bp_sb[:, lo:lo + qw],
                                mybir.AluOpType.add,
                            )
                        # final chunk: alternate the writeback DMAs over two
                        # queues (the ACT queue is drained of exps)
                        eng = (nc.scalar if k == NSC - 1 and (half + q) % 2
                               else nc.sync)
                        eng.dma_start(
                            out[k * QB:(k + 1) * QB, lo:lo + qw],
                            ot[:, lo:lo + qw])

            # ===== schedule =====
            # proj(k) slots into the PE stream late enough that AllToAll#k
            # has completed - no PE stall: proj(0)/proj(1) during chunk 2,
            # proj(2) mid-chunk 3, proj(3) at the end. Issue order = PE/ACT
            # interleave: each QKV block is issued under an exp-heavy
            # attention window where the PE would otherwise idle.
            load_xt(0, 0, split=2)
            load_xt(1, 0, split=2)
            qkv(0, 0)
            load_xt(0, 1)
            # Software-pipelined instance stream: att(b0,h0,sc+1) is hoisted
            # between att(b1,h0,sc) and att(b1,h1,sc) so the exp stream never
            # drains at a chunk boundary (pv PSUM rotation still fits in 2
            # buffers with this order).
            attention(0, 0, 0)
            qkv(1, 0)
            load_xt(1, 1)
            load_wp_piece()
            attention(0, 1, 0)
            load_wp_piece()
            stage(0, 0)
            for sc in range(NSC):
                attention(1, 0, sc)
                load_wp_piece(2)
                if sc + 1 < NSC:
                    qkv(0, sc + 1, "q")
                    attention(0, 0, sc + 1, phase="prefix")
                attention(1, 1, sc)
                load_wp_piece(2)
                if sc + 1 < NSC:
                    # K/V is only needed by the suffix's diagonal tiles, so
                    # it runs under att(b1,h1,sc)'s exp window instead of
                    # delaying it
                    qkv(0, sc + 1, "kv")
                    if sc + 2 < NSC:
                        load_xt(0, sc + 2)
                    attention(0, 0, sc + 1, phase="suffix")
                stage(1, sc)
                if sc == 3:
                    # proj(2)'s PE work runs while AllToAll#3 is in flight
                    proj(2)
                cc = a2a(sc)
                if sc + 1 < NSC:
                    attention(0, 1, sc + 1)
                    load_wp_piece(2)
                    qkv(1, sc + 1)
                    if sc + 2 < NSC:
                        load_xt(1, sc + 2)
                    stage(0, sc + 1)
                    if sc + 1 == 2:
                        proj(0)
                if sc == 2:
                    proj(1)
            # The PE idles ~18us during AllToAll#3; an idle PE drops to the
            # cold p-state and proj(3) would pay the DVFS ramp. Stream
            # dep-free throwaway matmuls through the gap: they start right
            # after proj(2) drains and keep pe_busy_start pinned so proj(3)
            # dispatches at the full 2.4GHz.
            warm_ps = psum.tile([P, SC], F32, tag="aux", name="warm_ps")
            for w in range(NWARM):
                nc.tensor.matmul(
                    warm_ps[:], wp_sb[:, 0, 0:P], wp_sb[:, 0, 0:SC],
                    start=(w == 0), stop=(w == NWARM - 1),
                )
            proj(3)
    return nc


_NC_CACHE = None


def _get_nc():
    global _NC_CACHE
    if _NC_CACHE is None:
        nc = bacc.Bacc("TRN2", target_bir_lowering=False, debug=False,
                       num_devices=N_CORES)
        build(nc)
        nc.compile()
        _NC_CACHE = nc
    return _NC_CACHE


def _fp8_pair(a):
    """Quantize float32 array -> (fp8, fp8 residual) pair."""
    fp8 = ml_dtypes.float8_e4m3
    a1 = a.astype(fp8)
    a2 = (a - a1.astype(np.float32)).astype(fp8)
    return np.ascontiguousarray(a1), np.ascontiguousarray(a2)


def _dr_layout(a):
    """[K, N] -> DoubleRow layout [p, ko2, i, N] with K = 256*ko2+128*i+p."""
    k, n = a.shape
    return np.ascontiguousarray(
        a.reshape(KO2, 2, P, n).transpose(2, 0, 1, 3))


def make_in_maps(x, c_attn_w, c_attn_b, c_proj_w, c_proj_b):
    x = np.asarray(x, dtype=np.float32)
    c_attn_w = np.asarray(c_attn_w, dtype=np.float32)
    c_attn_b = np.asarray(c_attn_b, dtype=np.float32)
    c_proj_w = np.asarray(c_proj_w, dtype=np.float32)
    c_proj_b = np.asarray(c_proj_b, dtype=np.float32)

    bf16 = ml_dtypes.bfloat16
    r = np.arange(P)[:, None]
    xcol = np.arange(P)[None, :]
    trim = (xcol >= r).astype(np.float32)

    # x^T per batch as fp8 residual pair in DoubleRow layout
    xT_pairs = []
    for b in range(B):
        x1, x2 = _fp8_pair(np.ascontiguousarray(x[b].T))
        xT_pairs.append((_dr_layout(x1), _dr_layout(x2)))

    wp_full = np.ascontiguousarray(c_proj_w).astype(bf16)
    wp8a, wp8b = _fp8_pair(c_proj_w * W_SCALE)
    wp8a, wp8b = _dr_layout(wp8a), _dr_layout(wp8b)
    bp_full = np.repeat(c_proj_b[None, :], P, axis=0).astype(np.float32).copy()

    in_maps = []
    for c in range(N_CORES):
        fsl = slice(c * FG, (c + 1) * FG)
        bq = c_attn_b[0 * NX:1 * NX][fsl]
        bk = c_attn_b[1 * NX:2 * NX][fsl]
        m = {
            "bqk": (W_SCALE * np.stack([bq, bk], axis=1)).astype(
                np.float32).copy(),
            "bv": (W_SCALE * np.repeat(c_attn_b[2 * NX:3 * NX][fsl][None, :],
                                       P, axis=0)).astype(bf16),
            "wp": wp_full,
            "wp8_1": wp8a,
            "wp8_2": wp8b,
            "bp": bp_full,
            "trim": trim.astype(bf16),
            "onesd": np.ones((1, P), dtype=np.float32),
            "bp32r": (W_SCALE * c_proj_b[None, :]).astype(np.float32).copy(),
        }
        for b in range(B):
            m[f"x1_{b}"], m[f"x2_{b}"] = xT_pairs[b]
        for nm, base in (("wq", 0), ("wk", 1), ("wv", 2)):
            w = np.ascontiguousarray(
                c_attn_w[:, base * NX:(base + 1) * NX][:, fsl]) * W_SCALE
            w1, w2 = _fp8_pair(w)
            m[f"{nm}1"] = _dr_layout(w1)
            m[f"{nm}2"] = _dr_layout(w2)
        in_maps.append(m)
    return in_maps


def assemble(results):
    """[core]{'out': [4*QB, NX]} -> [B, S, NX]; core c owns query rows
    [sc*SC + (c%4)*QB, +QB) of batch c//4 for each chunk sc."""
    full = np.empty((B, S, NX), dtype=np.float32)
    for c in range(N_CORES):
        b, r = divmod(c, 4)
        o = results[c]["out"]
        for k in range(NSC):
            full[b, k * SC + r * QB:k * SC + (r + 1) * QB, :] = \
                o[k * QB:(k + 1) * QB]
    return full


def kernel(x, c_attn_w, c_attn_b, c_proj_w, c_proj_b):
    nc = _get_nc()
    in_maps = make_in_maps(x, c_attn_w, c_attn_b, c_proj_w, c_proj_b)
    res = run_bass_kernel_spmd(nc, in_maps, core_ids=list(range(N_CORES)))
    return assemble(res.results)


# revision 33
# speedup vs baseline: 1.0103x; 1.0103x over previous
"""Distributed causal multi-head attention for 8 TRN2 NeuronCores.

Problem: y = (softmax(mask(Q K^T / sqrt(d))) V) @ c_proj_w + c_proj_b with
Q,K,V = split(x @ c_attn_w + c_attn_b), shapes B=2, S=2048, NX=1024, NH=16,
HD=64.

Sharding: tensor parallel 8-way over heads. Core c owns heads {2c, 2c+1}
(feature cols [128c, 128c+128) of each QKV block) and computes attention for
those 2 heads over BOTH batches. The attention outputs aT are then
redistributed with four per-chunk 8-core AllToAll collectives (one per
512-query chunk, fired as soon as that chunk's attention finishes on both
batches) so that core c ends up with ALL 1024 features for query rows
[sc*512 + (c%4)*128, +128) of batch c//4 per chunk sc; it then runs the
full output projection for those rows.

Numerics/performance strategy:
  * The QKV projection runs on the PE in fp8e4m3 DoubleRow perf mode
    (2 contraction rows per partition, 0.5 cycles per output column).
    Accuracy is preserved with residual pairs quantized on the host:
    x = x1 + x2, w = w1 + w2, computing the w1x1 + w2x1 + w1x2 cross
    terms (the dropped x2w2 term is ~0.1%). c_attn_w is host-scaled by
    W_SCALE=32 to lift its ~0.02-sigma weights out of e4m3's subnormal
    range; the scale is divided back out in the q/k psum evictions and
    cancels in softmax for v (the ones-column carries the same scale).
  * The final chunk's AllToAll payload is fp8 (the last rows average over
    the longest prefix => smallest magnitudes => cheapest fp8), and its
    output projection consumes that fp8 directly in DoubleRow mode with
    an fp8 (wp1 + wp2) residual pair - 4x fewer PE cycles on the tail.
  * Attention itself (scores exp(QK^T/8), PV) stays bf16: scores are
    64-deep contractions where DoubleRow wins nothing, and fp8 Q/K/P/V
    would blow the 2e-2 error budget.

Per-core attention (2 heads x 2 batches = 4 head instances):
  1. qT/kT ([d, s] layout) and v ([s, d] layout) computed from fp8-pair
     x^T, so no on-device transposes are needed;
  2. causal attention in the "S^T" orientation: scores come out of the PE
     as [j, i] tiles, exp() fused into the PSUM->SBUF copy on the scalar
     engine (no max-subtraction - scores are bounded), softmax denominator
     falls out of the PV matmul via a ones-column appended to V;
  3. the in-band causal mask is a single [128,128] lower-triangular
     multiply on the leading 128 columns of each diagonal score tile;
  4. normalization: reciprocal_approx_fast directly on the PV psum row
     (f32), bitcast to f32r for the PE ones-broadcast (1 cycle/col), DVE
     psum->sbuf bounce, final multiply on the DVE writing the staging
     tile. The Pool queue is NOT used mid-kernel: each collective parks
     the in-order Pool queue for its full 15-21us duration.

Scheduling notes:
  * Issue order = PE/ACT interleave strategy: the PE queue is in-order,
    so each QKV block is issued under an exp-heavy attention window where
    the PE would otherwise idle.
  * wp/bp/wp8 constants stream in as ko-sized pieces interleaved with the
    early schedule: a monolithic 6us DMA on any queue parks that queue.
  * ~88 dep-free throwaway matmuls stream through the AllToAll#3 gap so
    the PE's DVFS ramp (0.65/1.2/2.4GHz p-states) stays pinned at full
    speed for the final projection.
"""

import ml_dtypes
import numpy as np

import concourse.bass as bass
import concourse.mybir as mybir
from concourse import bacc, tile
from concourse.bass_utils import run_bass_kernel_spmd

B, S, NX, NH, HD = 2, 2048, 1024, 16, 64
HC = 2              # heads per core
FG = HC * HD        # local feature width (128)
P = 128
SC = 512            # sequence chunk width
NSC = S // SC       # 4 chunks
KO2 = 4             # 256-wide DoubleRow contraction tiles (4 x 256 = 1024)
KO = NX // P        # 8 contraction tiles (bf16 output projection)
W_SCALE = 32.0      # host premultiplier on c_attn_w (fp8e4m3 subnormal fix)
N_CORES = 8
QB = 128            # query block owned per core per AllToAll

F32 = mybir.dt.float32
MM_DT = mybir.dt.bfloat16
F32R = mybir.dt.float32r
FP8 = mybir.dt.float8e4
DR = mybir.MatmulPerfMode.DoubleRow

REPLICA_GROUPS = [[0, 1, 2, 3, 4, 5, 6, 7]]

# (label, first-instruction-id) checkpoints recorded during build; used by
# the dev-loop timing tools to attribute sim slices to kernel phases.
BUILD_TRACE = []

NWARM = 88          # p-state keeper matmuls through the AllToAll#3 gap


def build(nc: bass.Bass):
    # fp8 residual pairs, host-prequantized. Contraction index
    # k = 256*ko2 + 128*i + p for layout [p, ko2, i, ...].
    xs = [[nc.declare_dram_parameter(f"x{t}_{b}", [P, KO2, 2, S], FP8,
                                     isOutput=False)
           for t in (1, 2)] for b in range(B)]
    wq = [nc.declare_dram_parameter(f"wq{t}", [P, KO2, 2, FG], FP8,
                                    isOutput=False) for t in (1, 2)]
    wk = [nc.declare_dram_parameter(f"wk{t}", [P, KO2, 2, FG], FP8,
                                    isOutput=False) for t in (1, 2)]
    wv = [nc.declare_dram_parameter(f"wv{t}", [P, KO2, 2, FG], FP8,
                                    isOutput=False) for t in (1, 2)]
    wp = nc.declare_dram_parameter("wp", [NX, NX], MM_DT, isOutput=False)
    wp8 = [nc.declare_dram_parameter(f"wp8_{t}", [P, KO2, 2, NX], FP8,
                                     isOutput=False) for t in (1, 2)]
    bqk = nc.declare_dram_parameter("bqk", [P, 2], F32, isOutput=False)
    bv = nc.declare_dram_parameter("bv", [P, FG], MM_DT, isOutput=False)
    bp = nc.declare_dram_parameter("bp", [P, NX], F32, isOutput=False)
    trim = nc.declare_dram_parameter("trim", [P, P], MM_DT, isOutput=False)
    onesd = nc.declare_dram_parameter("onesd", [1, P], F32R, isOutput=False)
    bp32r = nc.declare_dram_parameter("bp32r", [1, NX], F32R, isOutput=False)
    out = nc.declare_dram_parameter("out", [NSC * QB, NX], F32, isOutput=True)

    # Collective bounce buffers (collectives can't touch kernel I/O).
    a2a_warm_in = nc.dram_tensor("a2a_warm_in", [8, 128], MM_DT)
    a2a_warm_out = nc.dram_tensor("a2a_warm_out", [8, 128], MM_DT)
    # the final chunk's exchange is on the critical path: fp8 payload
    a2a_dt = [MM_DT, MM_DT, MM_DT, FP8]
    a2a_in = [nc.dram_tensor(f"a2a_in{k}", [8, FG, QB], a2a_dt[k])
              for k in range(NSC)]
    a2a_out = [nc.dram_tensor(f"a2a_out{k}", [8, FG, QB], a2a_dt[k])
               for k in range(NSC)]

    with tile.TileContext(nc) as tc:
        nc_lp = nc.allow_low_precision(reason="fp8 DoubleRow compute path")
        nc_lp.__enter__()
        with (
            tc.tile_pool(name="consts", bufs=1) as consts,
            tc.tile_pool(name="persist", bufs=1) as persist,
            tc.tile_pool(name="xt", bufs=4) as xt_pool,
            tc.tile_pool(name="pt", bufs=12) as pt_pool,
            tc.tile_pool(name="aTf", bufs=2) as aTf_pool,
            tc.tile_pool(name="outs", bufs=3) as out_pool,
            tc.tile_pool(name="small", bufs=4) as small,
            tc.tile_pool(name="psum", bufs=2, space="PSUM") as psum,
        ):
            # ---- load weights / constants ----
            wq_sb = [consts.tile([P, KO2, 2, FG], FP8, name=f"wq{t}")
                     for t in range(2)]
            wk_sb = [consts.tile([P, KO2, 2, FG], FP8, name=f"wk{t}")
                     for t in range(2)]
            wv_sb = [consts.tile([P, KO2, 2, FG], FP8, name=f"wv{t}")
                     for t in range(2)]
            wp_sb = consts.tile([P, KO, NX], MM_DT)
            wp8_sb = [consts.tile([P, KO2, 2, NX], FP8, name=f"wp8_{t}")
                      for t in range(2)]
            bqk_sb = consts.tile([P, 2], F32)
            bv_sb = consts.tile([P, FG], MM_DT)
            bp_sb = consts.tile([P, NX], F32)
            tri_sb = consts.tile([P, P], MM_DT)
            ones128 = consts.tile([1, P], F32R)
            bp32_sb = consts.tile([1, NX], F32R)
            for t in range(2):
                nc.sync.dma_start(wq_sb[t][:], wq[t][:])
                nc.gpsimd.dma_start(wk_sb[t][:], wk[t][:])
                nc.gpsimd.dma_start(wv_sb[t][:], wv[t][:])
            nc.gpsimd.dma_start(bqk_sb[:], bqk[:])
            nc.gpsimd.dma_start(bv_sb[:], bv[:])
            nc.gpsimd.dma_start(tri_sb[:], trim[:])
            nc.gpsimd.dma_start(ones128[:], onesd[:])
            nc.gpsimd.dma_start(bp32_sb[:], bp32r[:])

            # wp/bp/wp8 stream in as pieces interleaved with the early
            # schedule (emitted via load_wp_piece below): a monolithic DMA
            # would park its queue for ~6us
            wp_src = wp.rearrange("(ko p) f -> p ko f", p=P)
            wp_piece = [0]

            def load_wp_piece(n=1):
                for _ in range(n):
                    k = wp_piece[0]
                    if k < KO:
                        nc.gpsimd.dma_start(wp_sb[:, k, :], wp_src[:, k, :])
                    elif k < KO + 2:
                        hb = k - KO
                        nc.gpsimd.dma_start(
                            bp_sb[:, hb * SC:(hb + 1) * SC],
                            bp[:, hb * SC:(hb + 1) * SC])
                    elif k < KO + 2 + 2 * KO2:
                        t, k2 = divmod(k - KO - 2, KO2)
                        nc.gpsimd.dma_start(wp8_sb[t][:, k2], wp8[t][:, k2])
                    wp_piece[0] += 1

            # ---- persistent activation tiles ----
            # kT[b]: [d, s] packed - head 0 on partitions 0:64, head 1 on
            # 64:128; the scores lhsT.
            # qT[2b+h]: zero-padded [128, s], data on the same partition half
            # as in kT - zeros select the head out of packed kT.
            # v[b]: [s-tile, st, h, d] with a ones column at col 64 per head
            # (softmax denominator, carrying W_SCALE so it cancels) and zero
            # pad to 128 cols so the PV lhsT is a full [128,128] block.
            # aT[2b+h]: [64, s]; the normalized attention output.
            qT_sb = [persist.tile([P, S], MM_DT, name=f"qT{i}") for i in range(4)]
            kT_sb = [persist.tile([P, S], MM_DT, name=f"kT{b}") for b in range(2)]
            v_sb = [persist.tile([P, S // P, HC, P], MM_DT, name=f"v{b}")
                    for b in range(2)]
            aT_sb = [persist.tile([P, S], MM_DT, name=f"aT{i}") for i in range(4)]
            # chunk-3 attention output in fp8 for the critical-path exchange
            aT8_sb = [persist.tile([P, SC], FP8, name=f"aT8_{i}")
                      for i in range(4)]
            # pads on the Pool engine (in b0-first order) so the DVE is free
            # for chunk-0 QKV evictions from the start
            for b in range(2):
                for h in range(HC):
                    i = 2 * b + h
                    pad0 = (1 - i % 2) * HD
                    nc.gpsimd.memset(qT_sb[i][pad0:pad0 + HD, :], 0.0)
                nc.gpsimd.memset(v_sb[b][:, :, :, HD:], 0.0)
                # the ones column carries the same W_SCALE as the v data
                # columns, so the softmax normalization cancels the scale
                nc.gpsimd.memset(v_sb[b][:, :, :, HD], W_SCALE)
            # warmup collective after the memsets (a collective blocks the
            # Pool queue for its full duration): establishes the channel and
            # absorbs the fixed collective latency off the critical path
            nc.gpsimd.collective_compute(
                "AllToAll",
                mybir.AluOpType.bypass,
                ins=[a2a_warm_in[:].opt()],
                outs=[a2a_warm_out[:].opt()],
                replica_groups=REPLICA_GROUPS,
            )

            xts = {}

            def mark(label):
                BUILD_TRACE.append((label, nc.next_id()))

            def load_xt(b, sc, split=1):
                mark(f"load_xt{b}_{sc}")
                cols = slice(sc * SC, (sc + 1) * SC)
                pair = []
                for t in range(2):
                    xt = xt_pool.tile([P, KO2, 2, SC], FP8, tag=f"x{t}",
                                      name=f"x{t}_{b}_{sc}")
                    kq = KO2 // split
                    for i in range(split):
                        ks = slice(i * kq, (i + 1) * kq)
                        nc.sync.dma_start(xt[:, ks, :, :],
                                          xs[b][t][:, ks, :, cols])
                    pair.append(xt)
                xts[(b, sc)] = pair

            def qkv(b, sc, part="all"):
                mark(f"qkv{b}_{sc}_{part}")
                x1t, x2t = xts[(b, sc)]
                cols = slice(sc * SC, (sc + 1) * SC)
                parts = {"all": (0, 1), "q": (0,), "kv": (1,),
                         "qk": (0, 1), "v": ()}[part]
                for qk in parts:
                    w_pair = (wq_sb, wk_sb)[qk]
                    terms = [(w_pair[0], x1t), (w_pair[1], x1t),
                             (w_pair[0], x2t)]
                    ps = psum.tile([P, SC], F32, tag="mm_ps", name="mm_ps")
                    n = len(terms) * KO2
                    m = 0
                    for w_sb, xt in terms:
                        for ko2 in range(KO2):
                            nc.tensor.matmul(
                                ps[:], w_sb[:, ko2], xt[:, ko2],
                                start=(m == 0), stop=(m == n - 1),
                                perf_mode=DR,
                            )
                            m += 1
                    if qk == 1:
                        # kT/qT stay scaled by W_SCALE; the 1/W_SCALE^2 on
                        # the scores is folded into the exp's scale argument
                        nc.vector.tensor_scalar_add(
                            kT_sb[b][:, cols], ps[:], bqk_sb[:, 1:2])
                    elif part == "q":
                        # boundary-critical eviction: the ACT engine is idle
                        # here (exp gap) while the DVE is deep in the previous
                        # instance's normalize chain; Identity shares the act
                        # table with Exp so there is no table-switch cost
                        for hr in range(HC):
                            rr = slice(hr * HD, (hr + 1) * HD)
                            nc.scalar.activation(
                                qT_sb[2 * b + hr][rr, cols], ps[rr, :],
                                mybir.ActivationFunctionType.Identity,
                                bias=bqk_sb[rr, 0:1],
                            )
                    else:
                        for hr in range(HC):
                            rr = slice(hr * HD, (hr + 1) * HD)
                            nc.vector.tensor_scalar_add(
                                qT_sb[2 * b + hr][rr, cols], ps[rr, :],
                                bqk_sb[rr, 0:1])
                if part in ("q", "qk"):
                    return
                terms = [(wv_sb[0], x1t), (wv_sb[1], x1t), (wv_sb[0], x2t)]
                for st in range(SC // P):
                    g_s = sc * (SC // P) + st
                    rows = slice(st * P, (st + 1) * P)
                    ps = psum.tile([P, SC], F32, tag="mm_ps", name="mm_ps")[:, :FG]
                    n = len(terms) * KO2
                    m = 0
                    for w_sb, xt in terms:
                        for ko2 in range(KO2):
                            nc.tensor.matmul(
                                ps[:],
                                xt[:, ko2, :, rows],
                                w_sb[:, ko2],
                                start=(m == 0), stop=(m == n - 1),
                                perf_mode=DR,
                            )
                            m += 1
                    for h in range(HC):
                        nc.vector.tensor_tensor(
                            v_sb[b][:, g_s, h, 0:HD],
                            ps[:, h * HD:(h + 1) * HD],
                            bv_sb[:, h * HD:(h + 1) * HD],
                            mybir.AluOpType.add,
                        )

            pv_carry = {}
            pt_carry = {}

            def attention(b, h, sc, phase="all"):
                mark(f"att{b}_{h}_{sc}_{phase[0]}{phase[-1]}")
                i = 2 * b + h
                n_j = (sc + 1) * (SC // P)
                cut = min(4, 4 * sc)
                if phase == "prefix":
                    # non-diagonal score tiles against already-resident K/V:
                    # only this chunk's Q is needed, so these exps can fill
                    # the ACT gap while the chunk's K/V matmuls still run
                    jts = range(0, cut)
                    pv = psum.tile([P, SC], F32, tag="pv", name="pv")
                    pv_carry[(b, h, sc)] = pv
                elif phase == "suffix":
                    jts = range(cut, n_j)
                    pv = pv_carry.pop((b, h, sc))
                elif phase == "exp0":
                    # scores+exps only (V not yet computed): pt tiles carried
                    jts = range(n_j)
                    pv = None
                elif phase == "pv0":
                    jts = range(n_j)
                    pv = psum.tile([P, SC], F32, tag="pv")
                else:
                    jts = range(n_j)
                    pv = psum.tile([P, SC], F32, tag="pv")
                for jt in jts:
                    o = jt - 4 * sc
                    off = max(0, 128 * o)  # diagonal blocks: skip i < j
                    if phase == "pv0":
                        pt = pt_carry.pop((i, jt))
                    else:
                        sp = psum.tile([P, SC], F32, tag="score", bufs=2)
                        nc.tensor.matmul(
                            sp[:, off:],
                            kT_sb[b][:, jt * P:(jt + 1) * P],
                            qT_sb[i][:, sc * SC + off:(sc + 1) * SC],
                            start=True, stop=True,
                        )
                        pt = pt_pool.tile([P, SC], MM_DT, tag="pt")
                        # exp(scores / sqrt(HD)); scores are bounded, no max
                        nc.scalar.activation(
                            pt[:, off:], sp[:, off:],
                            mybir.ActivationFunctionType.Exp,
                            scale=1.0 / float(np.sqrt(HD) * W_SCALE * W_SCALE),
                        )
                        if o >= 0:
                            # in-band causal mask on the diagonal block
                            nc.vector.tensor_tensor(
                                pt[:, off:off + P], pt[:, off:off + P],
                                tri_sb[:], mybir.AluOpType.mult,
                            )
                    if phase == "exp0":
                        pt_carry[(i, jt)] = pt
                        continue
                    nc.tensor.matmul(
                        pv[:, off:],
                        v_sb[b][:, jt, h, :],
                        pt[:, off:],
                        start=(jt == 0), stop=(jt == n_j - 1),
                    )
                if phase in ("prefix", "exp0"):
                    return
                # normalize: reciprocal of the ones-column row, broadcast
                # over partitions via a PE f32r ones-matmul, DVE psum->sbuf
                # bounce, final multiply evicting into the staging tile
                lrow = small.tile([1, SC], F32, tag="lrow")
                nc.vector.tensor_copy(lrow[:], pv[HD:HD + 1, :])
                rec = small.tile([1, SC], F32, tag="rec")
                nc.vector.reciprocal_approx_fast(rec[:], lrow[:])
                # the PE broadcast wants f32r operands and the BIR verifier
                # requires an explicit rounding op (a bitcast is rejected)
                rec_r = small.tile([1, SC], F32R, tag="rec_r")
                nc.vector.tensor_copy(rec_r[:], rec[:])
                rb = psum.tile([P, SC], F32, tag="aux", name="rb")
                nc.tensor.matmul(rb[:], ones128[:], rec_r[:],
                                 start=True, stop=True)
                rbs = small.tile([P, SC], F32, tag="rbs")
                nc.vector.tensor_copy(rbs[:], rb[:])
                dst = (aT8_sb[i][:, :] if sc == NSC - 1
                       else aT_sb[i][:, sc * SC:(sc + 1) * SC])
                nc.vector.tensor_tensor(
                    dst, pv[:], rbs[:], mybir.AluOpType.mult,
                )

            def stage(b, sc):
                mark(f"stage{b}_{sc}")
                # one DMA per head: DRAM-side AP iterates (d, r, q) so the
                # [64, 512] SBUF chunk scatters across the 4 rank blocks in
                # a single transfer (4 small DMAs pay ~2us of fixed DGE
                # overhead per stage on the SP queue)
                for h in range(HC):
                    src_t = aT8_sb[2 * b + h] if sc == NSC - 1 else aT_sb[2 * b + h]
                    base = 0 if sc == NSC - 1 else sc * SC
                    dst = a2a_in[sc][4 * b:4 * b + 4,
                                     h * HD:(h + 1) * HD, :].transpose([1, 0, 2])
                    src = src_t[0:HD, base:base + SC].rearrange(
                        "p (r q) -> p r q", r=4)
                    nc.sync.dma_start(dst, src)

            def a2a(k):
                mark(f"a2a{k}")
                return nc.gpsimd.collective_compute(
                    "AllToAll",
                    mybir.AluOpType.bypass,
                    ins=[a2a_in[k][:].opt()],
                    outs=[a2a_out[k][:].opt()],
                    replica_groups=REPLICA_GROUPS,
                )

            def proj(k):
                mark(f"proj{k}")
                aTf = aTf_pool.tile([P, KO, QB],
                                    FP8 if k == NSC - 1 else MM_DT,
                                    tag="aTf", name=f"aTf{k}")
                src = a2a_out[k].rearrange("ko p q -> p ko q")
                if k == NSC - 1:
                    # final proj: halve the load latency via two queues (the
                    # ACT queue is drained of exps by now)
                    nc.sync.dma_start(aTf[:, 0:KO // 2, :], src[:, 0:KO // 2, :])
                    nc.scalar.dma_start(aTf[:, KO // 2:, :], src[:, KO // 2:, :])
                else:
                    nc.sync.dma_start(aTf[:], src)
                ot = out_pool.tile([P, NX], F32, tag="ot")
                for half in range(2):
                    hs = slice(half * SC, (half + 1) * SC)
                    ps = psum.tile([P, SC], F32, tag="aux", name="proj_ps")
                    if k == NSC - 1:
                        # final chunk: the payload is already fp8, so run the
                        # projection in DoubleRow with the fp8 wp residual
                        # pair (host-scaled by W_SCALE) - 4x fewer PE cycles
                        # on the tail. The bias rides in as a ones-row f32r
                        # matmul so the eviction is a single 1/W_SCALE
                        # scalar multiply.
                        for t in range(2):
                            for ko2 in range(KO2):
                                nc.tensor.matmul(
                                    ps[:],
                                    aTf[:, 2 * ko2:2 * ko2 + 2, :],
                                    wp8_sb[t][:, ko2, :, hs],
                                    start=(t == 0 and ko2 == 0), stop=False,
                                    perf_mode=DR,
                                )
                        nc.tensor.matmul(ps[:], ones128[:], bp32_sb[:, hs],
                                         start=False, stop=True)
                    else:
                        for ko in range(KO):
                            nc.tensor.matmul(
                                ps[:],
                                aTf[:, ko, :],
                                wp_sb[:, ko, hs],
                                start=(ko == 0), stop=(ko == KO - 1),
                            )
                    # quarter the bias+writeback chain so the final DMA's
                    # completion latency hides behind the previous quarters
                    nq = 2 if k == NSC - 1 else 1
                    qw = SC // nq
                    for q in range(nq):
                        lo = half * SC + q * qw
                        if k == NSC - 1:
                            nc.vector.tensor_scalar_mul(
                                ot[:, lo:lo + qw], ps[:, q * qw:(q + 1) * qw],
                                1.0 / W_SCALE)
                        else:
                            nc.vector.tensor_tensor(
                                ot[:, lo:lo + qw], ps[:, q * qw:(q + 1) * qw],
                                bp_sb[:, lo:lo + qw],
                                mybir.AluOpType.add,
                            )
                        # final chunk: alternate the writeback DMAs over two
                        # queues (the ACT queue is drained of exps)
                        eng = (nc.scalar if k == NSC - 1 and (half + q) % 2
                               else nc.sync)
                        eng.dma_start(
                            out[k * QB:(k + 1) * QB, lo:lo + qw],
                            ot[:, lo:lo + qw])

            # ===== schedule =====
            # proj(k) slots into the PE stream late enough that AllToAll#k
            # has completed - no PE stall: proj(0)/proj(1) during chunk 2,
            # proj(2) mid-chunk 3, proj(3) at the end. Issue order = PE/ACT
            # interleave: each QKV block is issued under an exp-heavy
            # attention window where the PE would otherwise idle.
            load_xt(0, 0, split=2)
            load_xt(1, 0, split=2)
            qkv(0, 0)
            load_xt(0, 1)
            # Software-pipelined instance stream: att(b0,h0,sc+1) is hoisted
            # between att(b1,h0,sc) and att(b1,h1,sc) so the exp stream never
            # drains at a chunk boundary (pv PSUM rotation still fits in 2
            # buffers with this order).
            attention(0, 0, 0)
            qkv(1, 0)
            load_xt(1, 1)
            load_wp_piece()
            attention(0, 1, 0)
            load_wp_piece()
            stage(0, 0)
            for sc in range(NSC):
                attention(1, 0, sc)
                load_wp_piece(2)
                if sc + 1 < NSC:
                    qkv(0, sc + 1, "q")
                    attention(0, 0, sc + 1, phase="prefix")
                attention(1, 1, sc)
                load_wp_piece(2)
                if sc + 1 < NSC:
                    # K/V is only needed by the suffix's diagonal tiles, so
                    # it runs under att(b1,h1,sc)'s exp window instead of
                    # delaying it
                    qkv(0, sc + 1, "kv")
                    if sc + 2 < NSC:
                        load_xt(0, sc + 2)
                    attention(0, 0, sc + 1, phase="suffix")
                stage(1, sc)
                if sc == 3:
                    # proj(2)'s PE work runs while AllToAll#3 is in flight
                    proj(2)
                cc = a2a(sc)
                if sc + 1 < NSC:
                    attention(0, 1, sc + 1)
                    load_wp_piece(2)
                    qkv(1, sc + 1)
                    if sc + 2 < NSC:
                        load_xt(1, sc + 2)
                    stage(0, sc + 1)
                    if sc + 1 == 2:
                        proj(0)
                if sc == 2:
                    proj(1)
            # The PE idles ~18us during AllToAll#3; an idle PE drops to the
            # cold p-state and proj(3) would pay the DVFS ramp. Stream
            # dep-free throwaway matmuls through the gap: they start right
            # after proj(2) drains and keep pe_busy_start pinned so proj(3)
            # dispatches at the full 2.4GHz.
            warm_ps = psum.tile([P, SC], F32, tag="aux", name="warm_ps")
            for w in range(NWARM):
                nc.tensor.matmul(
                    warm_ps[:], wp_sb[:, 0, 0:P], wp_sb[:, 0, 0:SC],
                    start=(w == 0), stop=(w == NWARM - 1),
                )
            proj(3)
    return nc


_NC_CACHE = None


def _get_nc():
    global _NC_CACHE
    if _NC_CACHE is None:
        nc = bacc.Bacc("TRN2", target_bir_lowering=False, debug=False,
                       num_devices=N_CORES)
        build(nc)
        nc.compile()
        _NC_CACHE = nc
    return _NC_CACHE


def _fp8_pair(a):
    """Quantize float32 array -> (fp8, fp8 residual) pair."""
    fp8 = ml_dtypes.float8_e4m3
    a1 = a.astype(fp8)
    a2 = (a - a1.astype(np.float32)).astype(fp8)
    return np.ascontiguousarray(a1), np.ascontiguousarray(a2)


def _dr_layout(a):
    """[K, N] -> DoubleRow layout [p, ko2, i, N] with K = 256*ko2+128*i+p."""
    k, n = a.shape
    return np.ascontiguousarray(
        a.reshape(KO2, 2, P, n).transpose(2, 0, 1, 3))


def make_in_maps(x, c_attn_w, c_attn_b, c_proj_w, c_proj_b):
    x = np.asarray(x, dtype=np.float32)
    c_attn_w = np.asarray(c_attn_w, dtype=np.float32)
    c_attn_b = np.asarray(c_attn_b, dtype=np.float32)
    c_proj_w = np.asarray(c_proj_w, dtype=np.float32)
    c_proj_b = np.asarray(c_proj_b, dtype=np.float32)

    bf16 = ml_dtypes.bfloat16
    r = np.arange(P)[:, None]
    xcol = np.arange(P)[None, :]
    trim = (xcol >= r).astype(np.float32)

    # x^T per batch as fp8 residual pair in DoubleRow layout
    xT_pairs = []
    for b in range(B):
        x1, x2 = _fp8_pair(np.ascontiguousarray(x[b].T))
        xT_pairs.append((_dr_layout(x1), _dr_layout(x2)))

    wp_full = np.ascontiguousarray(c_proj_w).astype(bf16)
    wp8a, wp8b = _fp8_pair(c_proj_w * W_SCALE)
    wp8a, wp8b = _dr_layout(wp8a), _dr_layout(wp8b)
    bp_full = np.repeat(c_proj_b[None, :], P, axis=0).astype(np.float32).copy()

    in_maps = []
    for c in range(N_CORES):
        fsl = slice(c * FG, (c + 1) * FG)
        bq = c_attn_b[0 * NX:1 * NX][fsl]
        bk = c_attn_b[1 * NX:2 * NX][fsl]
        m = {
            "bqk": (W_SCALE * np.stack([bq, bk], axis=1)).astype(
                np.float32).copy(),
            "bv": (W_SCALE * np.repeat(c_attn_b[2 * NX:3 * NX][fsl][None, :],
                                       P, axis=0)).astype(bf16),
            "wp": wp_full,
            "wp8_1": wp8a,
            "wp8_2": wp8b,
            "bp": bp_full,
            "trim": trim.astype(bf16),
            "onesd": np.ones((1, P), dtype=np.float32),
            "bp32r": (W_SCALE * c_proj_b[None, :]).astype(np.float32).copy(),
        }
        for b in range(B):
            m[f"x1_{b}"], m[f"x2_{b}"] = xT_pairs[b]
        for nm, base in (("wq", 0), ("wk", 1), ("wv", 2)):
            w = np.ascontiguousarray(
                c_attn_w[:, base * NX:(base + 1) * NX][:, fsl]) * W_SCALE
            w1, w2 = _fp8_pair(w)
            m[f"{nm}1"] = _dr_layout(w1)
            m[f"{nm}2"] = _dr_layout(w2)
        in_maps.append(m)
    return in_maps


def assemble(results):
    """[core]{'out': [4*QB, NX]} -> [B, S, NX]; core c owns query rows
    [sc*SC + (c%4)*QB, +QB) of batch c//4 for each chunk sc."""
    full = np.empty((B, S, NX), dtype=np.float32)
    for c in range(N_CORES):
        b, r = divmod(c, 4)
        o = results[c]["out"]
        for k in range(NSC):
            full[b, k * SC + r * QB:k * SC + (r + 1) * QB, :] = \
                o[k * QB:(k + 1) * QB]
    return full


def kernel(x, c_attn_w, c_attn_b, c_proj_w, c_proj_b):
    nc = _get_nc()
    in_maps = make_in_maps(x, c_attn_w, c_attn_b, c_proj_w, c_proj_b)
    res = run_bass_kernel_spmd(nc, in_maps, core_ids=list(range(N_CORES)))
    return assemble(res.results)
